# revision 36
# baseline (speedup 1.0000x reference)
"""AutoCorrelation kernel for Trainium2 (Bass/Tile), 8-core data parallel.

Math: the reference computes rfft over the zero-padded head dim (D=64 -> L=512),
multiplies conj(Q)*K, irffts, then MEANS over heads AND the whole lag axis.
Summing a circular correlation over all lags factorizes:
    sum_t corr[t] = (sum_d q[d]) * (sum_d k[d])
so  x_corr_mean[b,l] = 1/(H*L) * sum_h (sum_d q[b,l,h,:]) * (sum_d k[b,l,h,:]).
Then top-6 over l per batch, softmax, weighted sum of values rows -> [B,H,D].

Sharding: batch 16 -> 2 per core across 8 cores, no cross-core communication.

Per core, two per-batch pipelines staggered so batch 0's tail hides under
batch 1's loads:
 - q pieces stream on the ACT HWDGE queue, k on SP (parallel issue/queues).
   Piece order: b0 small chunks first (so DVE starts ~2us earlier), b1 small
   chunk LAST (so the final reduce after the last byte is short).
 - DVE does ONLY the d-axis row-sum reduces plus the fused sq*sk+h-reduce
   (scalar_tensor_tensor accum_out); PSUM->SBUF copies run on ACT.
 - Each corr chunk column [128,1] is PE-transposed into a per-batch PSUM row.
 - Split top-k follows ARRIVAL order: the early-arriving lag region is
   searched while the late region is still in flight. (value, lag)
   candidates pack the 9-bit lag into the low mantissa bits; one MAX8 over
   [1,16] merges; the lag pops out with a bitwise AND.
 - One 32x32 stream transpose each moves the gather row-ids (bit-preserved)
   and the raw exp weights onto partitions; gather offsets must sit at
   partition base 0 (SWDGE requirement found the hard way).
 - Weighted sum = single-pass fp32r matmul with UNNORMALIZED exp weights;
   the softmax 1/sum is folded into the ACT copy out of PSUM; stores issue
   from ACT right after.
"""

import numpy as np

import concourse.bass as bass
import concourse.mybir as mybir
import concourse.tile as tile
from concourse.masks import make_identity
from concourse.bass_utils import run_bass_kernel_spmd

B, L, H, D = 16, 512, 8, 64
HD = H * D                  # 512
NCORES = 8
BPC = B // NCORES           # 2 batches per core
ROWS = BPC * L              # 1024 rows of [HD] per core
P = 128
TPB = L // P                # 4 chunks per batch
KTOP = 6                    # k = int(log(512)) = 6
SCALE = 1.0 / (H * L)

MASK_HI = 0xFFFFFE00        # clears the low 9 mantissa bits
MASK_LAG = 0x1FF

_CACHE = {}


def _emit(tc, q, k, v, out):
    nc = tc.nc
    from contextlib import ExitStack

    f32 = mybir.dt.float32
    u32 = mybir.dt.uint32
    f32r = mybir.dt.float32r
    AX = mybir.AxisListType.X
    AluOp = mybir.AluOpType

    with ExitStack() as ctx:
        main = ctx.enter_context(tc.tile_pool(name="main", bufs=1))
        psum = ctx.enter_context(tc.tile_pool(name="psum", bufs=1, space="PSUM"))

        # ---- constants (gpsimd, off the critical path) ----
        ident = main.tile([P, P], f32)
        make_identity(nc, ident[:])
        maskc8 = main.tile([1, 8], u32)
        nc.gpsimd.memset(maskc8[:], MASK_HI)
        c1ff8 = main.tile([1, 8], u32)
        nc.gpsimd.memset(c1ff8[:], MASK_LAG)
        orbase = {}
        for base in (2 * P, 3 * P, 4 * P):
            t_ = main.tile([1, 8], u32, tag=f"or{base}")
            nc.gpsimd.memset(t_[:], base)
            orbase[base] = t_
        stages = {}
        idstages = {}
        for b in range(BPC):
            s_ = main.tile([32, 32], f32, tag=f"stage{b}")
            nc.gpsimd.memset(s_[:], 0.0)
            stages[b] = s_
            i_ = main.tile([32, 32], f32, tag=f"idstage{b}")
            nc.gpsimd.memset(i_[:], 0.0)
            idstages[b] = i_

        q3 = q.rearrange("(t p) m -> t p m", p=P)
        k3 = k.rearrange("(t p) m -> t p m", p=P)

        # ---- loads ----
        # b0: c3, c2, then big(c0+c1).  b1: big, c2, then c3.
        qt, kt = {}, {}

        def load_piece(b, which):
            t0 = b * TPB
            if which == "big":
                qd = main.tile([P, 2, HD], f32, tag=f"qb{b}")
                nc.scalar.dma_start(
                    out=qd[:], in_=q3[t0 : t0 + 2].rearrange("t p m -> p t m")
                )
                kd = main.tile([P, 2, HD], f32, tag=f"kb{b}")
                nc.sync.dma_start(
                    out=kd[:], in_=k3[t0 : t0 + 2].rearrange("t p m -> p t m")
                )
            else:
                c = which
                qd = main.tile([P, HD], f32, tag=f"qc{c}_{b}")
                nc.scalar.dma_start(out=qd[:], in_=q3[t0 + c])
                kd = main.tile([P, HD], f32, tag=f"kc{c}_{b}")
                nc.sync.dma_start(out=kd[:], in_=k3[t0 + c])
            qt[(b, which)] = qd
            kt[(b, which)] = kd

        for b, which in ((0, "big"), (0, 2), (0, 3), (1, "big"), (1, 2), (1, 3)):
            load_piece(b, which)

        st = {}
        for b in range(BPC):
            sq = main.tile([P, TPB * H], f32, tag=f"sq{b}")
            sk = main.tile([P, TPB * H], f32, tag=f"sk{b}")
            corr = main.tile([P, TPB], f32, tag=f"corr{b}")
            psumRow = psum.tile([1, L], f32, tag=f"psumRow{b}")
            row = main.tile([1, L], f32, tag=f"row{b}")
            cand = main.tile([1, 16], f32, tag=f"cand{b}")
            st[b] = {"sq": sq, "sk": sk, "corr": corr, "psumRow": psumRow,
                     "row": row, "cand": cand}

        def chunk_tail(b, c):
            """fused sq*sk + h-reduce on DVE, then PE-transpose the corr
            column into the per-batch PSUM row."""
            s = st[b]
            junk = main.tile([P, H], f32, tag=f"junk{b}_{c}")
            nc.vector.scalar_tensor_tensor(
                out=junk[:],
                in0=s["sq"][:, c * H : (c + 1) * H],
                scalar=1.0,
                in1=s["sk"][:, c * H : (c + 1) * H],
                op0=AluOp.mult,
                op1=AluOp.mult,
                accum_out=s["corr"][:, c : c + 1],
            )
            nc.tensor.transpose(
                out=s["psumRow"][0:1, c * P : (c + 1) * P],
                in_=s["corr"][:, c : c + 1],
                identity=ident[:],
            )

        def red_small(b, c):
            s = st[b]
            nc.vector.reduce_sum(
                out=s["sq"][:, c * H : (c + 1) * H],
                in_=qt[(b, c)][:].rearrange("p (h d) -> p h d", d=D),
                axis=AX,
            )
            nc.vector.reduce_sum(
                out=s["sk"][:, c * H : (c + 1) * H],
                in_=kt[(b, c)][:].rearrange("p (h d) -> p h d", d=D),
                axis=AX,
            )
            chunk_tail(b, c)

        def red_big(b):
            s = st[b]
            nc.vector.reduce_sum(
                out=s["sq"][:, 0 : 2 * H],
                in_=qt[(b, "big")][:].rearrange("p t (h d) -> p (t h) d", d=D),
                axis=AX,
            )
            nc.vector.reduce_sum(
                out=s["sk"][:, 0 : 2 * H],
                in_=kt[(b, "big")][:].rearrange("p t (h d) -> p (t h) d", d=D),
                axis=AX,
            )
            chunk_tail(b, 0)
            chunk_tail(b, 1)

        def embed(cand_u_slice, vals, idx, extra_or=None, eng=None):
            """cand = (vals & MASK_HI) [| region base] | idx."""
            eng = eng or nc.vector
            eng.tensor_tensor(
                out=cand_u_slice,
                in0=vals[:].bitcast(u32),
                in1=maskc8[:],
                op=AluOp.bitwise_and,
            )
            if extra_or is not None:
                eng.tensor_tensor(
                    out=cand_u_slice,
                    in0=cand_u_slice,
                    in1=extra_or[:],
                    op=AluOp.bitwise_or,
                )
            eng.tensor_tensor(
                out=cand_u_slice,
                in0=cand_u_slice,
                in1=idx[:],
                op=AluOp.bitwise_or,
            )

        def topk_region(b, lo, hi, cslot, tag, embed_eng=None):
            """top-8 of lag region [lo*128, hi*128) -> embedded candidates."""
            s = st[b]
            nc.scalar.copy(
                s["row"][:, lo * P : hi * P], s["psumRow"][:, lo * P : hi * P]
            )
            mx = main.tile([1, 8], f32, tag=f"mx{tag}")
            ix = main.tile([1, 8], u32, tag=f"ix{tag}")
            nc.vector.max(out=mx[:], in_=s["row"][:, lo * P : hi * P])
            nc.vector.max_index(
                out=ix[:], in_max=mx[:], in_values=s["row"][:, lo * P : hi * P]
            )
            embed(
                s["cand"][:].bitcast(u32)[:, cslot : cslot + 8],
                mx,
                ix,
                extra_or=orbase[lo * P] if lo > 0 else None,
                eng=embed_eng,
            )

        def merge_gather(b):
            """merge candidates, stage ids+weights, launch the gather."""
            s = st[b]
            stage = stages[b]
            idstage = idstages[b]
            maxM = main.tile([1, 8], f32, tag=f"maxM{b}")
            nc.vector.max(out=maxM[:], in_=s["cand"][:])

            iu = idstage[:].bitcast(u32)
            nc.vector.tensor_tensor(
                out=iu[0:1, 0:8],
                in0=maxM[:].bitcast(u32),
                in1=c1ff8[:],
                op=AluOp.bitwise_and,
            )
            if b > 0:
                nc.vector.tensor_tensor(
                    out=iu[0:1, 0:8],
                    in0=iu[0:1, 0:8],
                    in1=orbase[4 * P][:],
                    op=AluOp.bitwise_or,
                )
            sm = main.tile([1, 1], f32, tag=f"sm{b}")
            nc.scalar.activation(
                out=stage[0:1, 0:KTOP],
                in_=maxM[:, 0:KTOP],
                func=mybir.ActivationFunctionType.Exp,
                scale=SCALE,
            )
            nc.vector.reduce_sum(out=sm[:], in_=stage[0:1, 0:KTOP], axis=AX)
            rs = main.tile([1, 1], f32, tag=f"rs{b}")
            nc.vector.reciprocal(out=rs[:], in_=sm[:])
            idstageT = main.tile([32, 32], f32, tag=f"idstageT{b}")
            nc.vector.transpose(out=idstageT[:], in_=idstage[:])
            stageT = main.tile([32, 32], f32, tag=f"stageT{b}")
            nc.vector.transpose(out=stageT[:], in_=stage[:])
            wcol = main.tile([KTOP, 1], f32r, tag=f"wcol{b}")
            nc.vector.tensor_copy(wcol[:], stageT[0:KTOP, 0:1])

            gath = main.tile([8, HD], f32r, tag=f"gath{b}")
            nc.gpsimd.indirect_dma_start(
                out=gath[:],
                out_offset=None,
                in_=v,
                in_offset=bass.IndirectOffsetOnAxis(
                    ap=idstageT[:].bitcast(u32)[0:8, 0:1], axis=0
                ),
            )
            st[b]["gath"] = gath
            st[b]["wcol"] = wcol
            st[b]["rs"] = rs

        def matmul_b(b):
            s = st[b]
            acc = psum.tile([1, HD], f32, tag=f"acc{b}")
            nc.tensor.matmul(
                out=acc[:],
                lhsT=s["wcol"][:],
                rhs=s["gath"][0:KTOP, :],
                start=True,
                stop=True,
            )
            s["acc"] = acc

        def copy_store(b):
            s = st[b]
            outt = main.tile([1, HD], f32, tag=f"outt{b}")
            nc.scalar.activation(
                out=outt[:],
                in_=s["acc"][:],
                func=mybir.ActivationFunctionType.Copy,
                scale=s["rs"][:, 0:1],
            )
            nc.scalar.dma_start(out=out[b : b + 1, :], in_=outt[:])

        # ---- staggered schedule following arrival order ----
        red_big(0)
        red_small(0, 2)
        red_small(0, 3)
        topk_region(0, 0, 3, 0, "b0early")
        topk_region(0, 3, 4, 8, "b0late")
        merge_gather(0)
        red_big(1)
        red_small(1, 2)
        topk_region(1, 0, 3, 0, "b1early")
        matmul_b(0)
        copy_store(0)
        red_small(1, 3)
        topk_region(1, 3, 4, 8, "b1late")   # chunk 3 lands last
        merge_gather(1)
        matmul_b(1)
        copy_store(1)


def _build_bass():
    import concourse.bacc as bacc

    nc = bacc.Bacc(trn_type="TRN2", target_bir_lowering=False, debug=False)
    q = nc.dram_tensor("q", [ROWS, HD], mybir.dt.float32, kind="ExternalInput").ap()
    k = nc.dram_tensor("k", [ROWS, HD], mybir.dt.float32, kind="ExternalInput").ap()
    v = nc.dram_tensor("v", [ROWS, HD], mybir.dt.float32, kind="ExternalInput").ap()
    out = nc.dram_tensor(
        "out", [BPC, HD], mybir.dt.float32, kind="ExternalOutput"
    ).ap()
    with tile.TileContext(nc) as tc:
        _emit(tc, q, k, v, out)
    nc.compile()
    return nc


def _get_nc():
    if "nc" not in _CACHE:
        _CACHE["nc"] = _build_bass()
    return _CACHE["nc"]


def run_sharded(queries, keys, values, trace=False, **kw):
    """Shard over 8 cores, run, gather. Returns (out [16,8,64], BassKernelResults)."""
    nc = _get_nc()
    q = np.ascontiguousarray(np.asarray(queries, dtype=np.float32))
    k = np.ascontiguousarray(np.asarray(keys, dtype=np.float32))
    v = np.ascontiguousarray(np.asarray(values, dtype=np.float32))
    in_maps = []
    for c in range(NCORES):
        sl = slice(c * BPC, (c + 1) * BPC)
        in_maps.append(
            {
                "q": q[sl].reshape(ROWS, HD),
                "k": k[sl].reshape(ROWS, HD),
                "v": v[sl].reshape(ROWS, HD),
            }
        )
    res = run_bass_kernel_spmd(nc, in_maps, list(range(NCORES)), trace=trace, **kw)
    out = np.empty((B, H, D), dtype=np.float32)
    for c in range(NCORES):
        out[c * BPC : (c + 1) * BPC] = res.results[c]["out"].reshape(BPC, H, D)
    return out, res


def kernel(queries, keys, values, B=None, **_ignored):
    out, _ = run_sharded(queries, keys, values, trace=False)
    return out
